# revision 15
# baseline (speedup 1.0000x reference)
"""CapsNet FOOD101 Trainium2 kernel (8 NeuronCores, SPMD).

Strategy
--------
Phase A (per-core batch-parallel, 4 samples/core):
  conv1 (11x11 s3) + ReLU and primary-caps conv (9x9 s3) run as direct
  convolutions on the PE: channels on partitions, strided SBUF access
  patterns supply the shifted windows (no im2col materialization).
Phase B: squash -> u (fp16), AllGather u across the 8 cores.
Phase C/D (o-parallel: 101 output caps padded to 104 = 8 cores x 13):
  x_hat[b,o,i,v] for the local 13 o's is built with block-diagonal-u
  matmuls (16 capsules x 8 dims = K=128) and kept RESIDENT in SBUF in
  fp16, [128 part = (i_l,b), free = (blk, o, v)], for a group of 8
  samples at a time (4 groups). All three routing iterations run
  on-chip: b-updates via DVE mult + tree reduction, softmax-over-o via
  ACT exp + a tiny cross-core AllReduce of the per-(b,i) denominators,
  and the s-contractions (sum over i) as PE matmuls with a masked-c
  stationary operand. dense_w is therefore read once per sample-group
  (4x 8.6MB fp16 per core) and x_hat never touches HBM.

All matmul operands are fp16 (PSUM accumulates fp32): simulated
end-to-end error vs the fp32 reference is ~4.4e-4 (scale-relative).
"""

import numpy as np
from contextlib import ExitStack

import concourse.bass as bass
import concourse.mybir as mybir
import concourse.tile as tile
from concourse import bacc
from concourse.bass_utils import run_bass_kernel_spmd

F32 = mybir.dt.float32
F16 = mybir.dt.float16
AX = mybir.AxisListType
ALU = mybir.AluOpType
ACTF = mybir.ActivationFunctionType

NC = 8            # cores
B = 32            # global batch
BL = B // NC      # local batch for conv phase (4)
OL = 13           # local output caps (101 padded to 104 = 8*13)
V = 16            # out-caps dim
OV = OL * V       # 208
NI = 2592         # primary caps
D = 8             # primary dim
IL = 16           # capsules per block (IL*D = 128)
NBLK = NI // IL   # 162
BG = 8            # samples per routing group
NBG = B // BG     # 4
IMG = 112 * 112   # 12544
IMGP = IMG + 16   # padded flat image
XW = 102 * 112 + 10  # replicated-row window length (11434)


def _build():
    nc = bacc.Bacc("TRN2", target_bir_lowering=False, debug=False,
                   num_devices=NC)

    # ---- external inputs ----
    x_d = nc.dram_tensor("x", [BL, 3, IMGP], F16, kind="ExternalInput").ap()
    w1_d = nc.dram_tensor("w1", [11, 33, 256], F16, kind="ExternalInput").ap()
    b1_d = nc.dram_tensor("b1", [256], F32, kind="ExternalInput").ap()
    w2_d = nc.dram_tensor("w2", [81, 256, 256], F16, kind="ExternalInput").ap()
    b2_d = nc.dram_tensor("b2", [256], F32, kind="ExternalInput").ap()
    wd_d = nc.dram_tensor("wd", [NBLK, 128, OV], F16, kind="ExternalInput").ap()
    maskd_d = nc.dram_tensor("maskd", [128, 128], F16, kind="ExternalInput").ap()
    maskb_d = nc.dram_tensor("maskb", [128, 8 * OL], F16, kind="ExternalInput").ap()
    ident_d = nc.dram_tensor("ident", [128, 128], F16, kind="ExternalInput").ap()
    dbias_d = nc.dram_tensor("dbias", [128, 1], F32, kind="ExternalInput").ap()

    v_out = nc.dram_tensor("v_out", [B, OL, V], F32, kind="ExternalOutput").ap()

    # ---- internal dram ----
    p_dram = nc.dram_tensor("p_dram", [BL, NI * D], F32).ap()
    u_loc = nc.dram_tensor("u_loc", [BL, NI * D], F16).ap()
    u_full = nc.dram_tensor("u_full", [B, NI * D], F16, addr_space="Shared").ap()
    v0_dram = nc.dram_tensor("v0_dram", [B, OV], F16).ap()
    v1_dram = nc.dram_tensor("v1_dram", [BG * OL, V], F16).ap()
    dl_dram = nc.dram_tensor("dl_dram", [128, NBLK], F32).ap()
    dg_dram = nc.dram_tensor("dg_dram", [128, NBLK], F32, addr_space="Shared").ap()

    with tile.TileContext(nc) as tc, ExitStack() as octx:
        gpool = octx.enter_context(tc.tile_pool(name="gl", bufs=1))
        eps_t = gpool.tile([128, 1], F32, tag="eps")
        nc.vector.memset(eps_t[:], 1e-8)

        # ======================= Phase A: convs =======================
        with ExitStack() as ctx:
            cpool = ctx.enter_context(tc.tile_pool(name="conv", bufs=2))
            cps = ctx.enter_context(tc.tile_pool(name="convps", bufs=3, space="PSUM"))
            pps = ctx.enter_context(tc.tile_pool(name="pcps", bufs=1, space="PSUM"))

            # conv1 weights: [33 (ci,ky), 11 kx, 256 co]
            w1_t = cpool.tile([33, 11, 256], F16, tag="w1", bufs=1)
            nc.sync.dma_start(w1_t[:], w1_d.transpose([1, 0, 2]))
            b1_t = cpool.tile([128, 1, 2], F32, tag="b1", bufs=1)
            nc.sync.dma_start(
                b1_t[:], b1_d.rearrange("(c p) -> p c", c=2).unsqueeze(1))
            b2_t = cpool.tile([128, 1, 2], F32, tag="b2", bufs=1)
            nc.sync.dma_start(
                b2_t[:], b2_d.rearrange("(c p) -> p c", c=2).unsqueeze(1))

            # h: conv1 output, [co(2x128), (b, 34, 34)] fp16
            # 36x36 alloc: pconv windows slice up to row/col 35; cells
            # >=34 are never read (max index 32) so they stay garbage.
            h_t = [cpool.tile([128, BL, 36, 36], F16, tag=f"h{cc}", bufs=1,
                              name=f"h{cc}")
                   for cc in range(2)]

            oy_chunks = [(0, 12), (12, 24), (24, 34)]
            for half in range(2):       # two samples at a time
                xr = cpool.tile([33, 2, XW], F16, tag="xr", bufs=2)
                for ky in range(11):
                    dst = xr[:].rearrange("(c k) b f -> c k b f", k=11)[:, ky]
                    src = x_d[2 * half:2 * half + 2].transpose([1, 0, 2])[
                        :, :, ky * 112: ky * 112 + XW]
                    nc.sync.dma_start(dst, src)
                for b2i in range(2):
                    b_abs = 2 * half + b2i
                    for cc in range(2):
                        for (y0, y1) in oy_chunks:
                            ps = cps.tile([128, 34 * 12], F32, tag="c1ps")
                            n = (y1 - y0) * 34
                            for kx in range(11):
                                rhs = (xr[:, b2i, kx: kx + 102 * 112]
                                       .rearrange("p (h w) -> p h w", w=112)
                                       [:, :, 0:102]
                                       .rearrange("p (a s) (c t) -> p a s c t",
                                                  s=3, t=3)[:, y0:y1, 0, :, 0])
                                nc.tensor.matmul(
                                    ps[:, 0:n],
                                    w1_t[:, kx, 128 * cc:128 * (cc + 1)],
                                    rhs,
                                    start=(kx == 0), stop=(kx == 10))
                            nc.scalar.activation(
                                h_t[cc][:, b_abs, y0:y1, 0:34], ps[:, 0:n],
                                ACTF.Relu, bias=b1_t[:, :, cc])

            # pconv: accumulate over (ky,kx,ci_chunk); out [co(2x128), (b,9,9)]
            pp = [pps.tile([128, BL * 81], F32, tag=f"pp{cc}", name=f"pp{cc}")
                  for cc in range(2)]
            first = True
            for ky in range(9):
                for kx in range(9):
                    for ci in range(2):
                        w2_t = cpool.tile([128, 256], F16, tag="w2", bufs=3)
                        nc.sync.dma_start(w2_t[:], w2_d[9 * ky + kx,
                                                        128 * ci:128 * (ci + 1)])
                        rhs = (h_t[ci][:, :, ky:ky + 27, kx:kx + 27]
                               .rearrange("p b (a s) (c t) -> p b a s c t",
                                          s=3, t=3)[:, :, :, 0, :, 0])
                        for cc in range(2):
                            nc.tensor.matmul(
                                pp[cc][:],
                                w2_t[:, 128 * cc:128 * (cc + 1)],
                                rhs,
                                start=first,
                                stop=(ky == 8 and kx == 8 and ci == 1))
                        first = False
            for cc in range(2):
                p_sb = cpool.tile([128, BL * 81], F32, tag="psb", bufs=2)
                nc.scalar.activation(p_sb[:], pp[cc][:], ACTF.Identity,
                                     bias=b2_t[:, :, cc])
                nc.sync.dma_start(
                    p_dram.rearrange("b (c q) -> c b q", c=256)
                    [128 * cc:128 * (cc + 1)],
                    p_sb[:].rearrange("p (b q) -> p b q", q=81))

        # ======================= Phase B: squash + allgather ==========
        with ExitStack() as ctx:
            spool = ctx.enter_context(tc.tile_pool(name="sq", bufs=1))
            p_t = spool.tile([108, 24, BL, D], F32, tag="pt")
            for b in range(BL):
                nc.sync.dma_start(
                    p_t[:, :, b, :],
                    p_dram[b].rearrange("(c p d) -> p c d", c=24, d=D))
            pf = p_t[:].rearrange("p c b d -> p (c b d)")
            p2 = spool.tile([108, 24 * BL * D], F32, tag="p2")
            nc.vector.tensor_tensor(p2[:], pf, pf, op=ALU.mult)
            sq = spool.tile([108, 24 * BL], F32, tag="sq")
            nc.vector.tensor_reduce(
                sq[:], p2[:].rearrange("p (g d) -> p g d", d=D), AX.X, ALU.add)
            t1 = spool.tile([108, 24 * BL], F32, tag="t1")
            nc.scalar.activation(t1[:], sq[:], ACTF.Sqrt, bias=eps_t[0:108])
            pl = spool.tile([108, 24 * BL], F32, tag="pl")
            nc.vector.tensor_scalar_add(pl[:], sq[:], 1.0)
            den = spool.tile([108, 24 * BL], F32, tag="den")
            nc.vector.tensor_tensor(den[:], pl[:], t1[:], op=ALU.mult)
            rcp = spool.tile([108, 24 * BL], F32, tag="rcp")
            nc.vector.reciprocal(rcp[:], den[:])
            fac = spool.tile([108, 24 * BL], F32, tag="fac")
            nc.vector.tensor_tensor(fac[:], sq[:], rcp[:], op=ALU.mult)
            u_t = spool.tile([108, 24, BL, D], F16, tag="ut")
            nc.vector.tensor_tensor(
                u_t[:], p_t[:],
                fac[:].rearrange("p (c b) -> p c b", c=24)
                .unsqueeze(3).to_broadcast((108, 24, BL, D)),
                op=ALU.mult)
            for b in range(BL):
                nc.sync.dma_start(
                    u_loc[b].rearrange("(c p d) -> p c d", c=24, d=D),
                    u_t[:, :, b, :])

            nc.gpsimd.collective_compute(
                "AllGather", ALU.bypass,
                replica_groups=[list(range(NC))],
                ins=[u_loc.opt()], outs=[u_full.opt()])

        # ======================= Phase C/D: routing ===================
        rp = octx.enter_context(tc.tile_pool(name="rt", bufs=1))
        wdp = octx.enter_context(tc.tile_pool(name="wdp", bufs=3))
        ubp = octx.enter_context(tc.tile_pool(name="ubp", bufs=2))
        trp = octx.enter_context(tc.tile_pool(name="trp", bufs=2))
        blp = octx.enter_context(tc.tile_pool(name="blp", bufs=2))
        mmps = octx.enter_context(tc.tile_pool(name="mmps", bufs=2, space="PSUM"))
        s0ps = octx.enter_context(tc.tile_pool(name="s0ps", bufs=1, space="PSUM"))
        sps = octx.enter_context(tc.tile_pool(name="sps", bufs=1, space="PSUM"))

        u_all = rp.tile([128, B, NBLK], F16, tag="uall")
        nc.sync.dma_start(
            u_all[:].rearrange("q b k -> q (b k)"),
            u_full.rearrange("b (k q) -> (b k) q", q=128),
            transpose=True)
        maskd_t = rp.tile([128, 128], F16, tag="maskd")
        nc.sync.dma_start(maskd_t[:], maskd_d)
        maskb_t = rp.tile([128, 8 * OL], F16, tag="maskb")
        nc.sync.dma_start(maskb_t[:], maskb_d)
        ident_t = rp.tile([128, 128], F16, tag="ident")
        nc.sync.dma_start(ident_t[:], ident_d)
        dbias_t = rp.tile([128, 1], F32, tag="dbias")
        nc.sync.dma_start(dbias_t[:], dbias_d)

        xh = rp.tile([128, NBLK, OV], F16, tag="xh")
        ps_s0a = s0ps.tile([128, B], F32, tag="s0a")
        ps_s0b = s0ps.tile([80, B], F32, tag="s0b")

        NTCH = 21  # b-pass tree chunk (blocks)
        tree_chunks = []
        st = 0
        while st < NBLK:
            tree_chunks.append((st, min(st + NTCH, NBLK)))
            st += NTCH

        for g in range(NBG):
            # blk-major copy of this group's u (so the ubd build gets a
            # packed innermost dim and the 2x DVE mode)
            u_g = rp.tile([128, NBLK, BG], F16, tag="ug")
            nc.vector.tensor_copy(
                u_g[:], u_all[:, BG * g:BG * (g + 1), :].transpose([0, 2, 1]))
            # ---- build block-diag u (supertiles of 18 blocks) ----
            ubds = []
            for sti in range(9):
                ubd = ubp.tile([128, 18, 128], F16, tag="ubd")
                nc.vector.tensor_tensor(
                    ubd[:].rearrange("p k (i b) -> p k i b", b=BG),
                    u_g[:, 18 * sti:18 * (sti + 1), :]
                    .unsqueeze(2).to_broadcast((128, 18, IL, BG)),
                    maskd_t[:].rearrange("p (i b) -> p i b", b=BG)
                    .unsqueeze(1).to_broadcast((128, 18, IL, BG)),
                    op=ALU.mult)
                ubds.append(ubd)

            # ---- create x_hat (and s0 partials on g==0) ----
            for blk in range(NBLK):
                wd_t = wdp.tile([128, OV], F16, tag="wd")
                nc.sync.dma_start(wd_t[:], wd_d[blk])
                psx = mmps.tile([128, OV], F32, tag="mmx")
                nc.tensor.matmul(psx[:], ubds[blk // 18][:, blk % 18],
                                 wd_t[:], start=True, stop=True)
                if blk % 2 == 0:
                    nc.scalar.copy(xh[:, blk], psx[:])
                else:
                    nc.vector.tensor_copy(xh[:, blk], psx[:])
                if g == 0:
                    nc.tensor.matmul(ps_s0a[:], wd_t[:, 0:128],
                                     u_all[:, :, blk],
                                     start=(blk == 0), stop=(blk == NBLK - 1))
                    nc.tensor.matmul(ps_s0b[:], wd_t[:, 128:OV],
                                     u_all[:, :, blk],
                                     start=(blk == 0), stop=(blk == NBLK - 1))

            if g == 0:
                # ---- s0 -> v0 for ALL 32 samples ----
                s0a = rp.tile([128, B], F16, tag="s0a_sb")
                nc.scalar.activation(s0a[:], ps_s0a[:], ACTF.Copy,
                                     scale=1.0 / 101.0)
                s0b = rp.tile([80, B], F16, tag="s0b_sb")
                nc.scalar.activation(s0b[:], ps_s0b[:], ACTF.Copy,
                                     scale=1.0 / 101.0)
                pt1 = sps.tile([B, 128], F16, tag="pt1")
                nc.tensor.transpose(pt1[:], s0a[:], ident_t[:])
                pt2 = sps.tile([B, 80], F16, tag="pt2")
                nc.tensor.transpose(pt2[:], s0b[:], ident_t[0:80, 0:80])
                s0T = rp.tile([B, OV], F32, tag="s0T")
                nc.scalar.copy(s0T[:, 0:128], pt1[:])
                nc.scalar.copy(s0T[:, 128:OV], pt2[:])
                # squash (per (b,o) over v)
                sq0 = rp.tile([B, OL], F32, tag="sq0")
                p20 = rp.tile([B, OV], F32, tag="p20")
                nc.vector.tensor_tensor(p20[:], s0T[:], s0T[:], op=ALU.mult)
                nc.vector.tensor_reduce(
                    sq0[:], p20[:].rearrange("p (o v) -> p o v", v=V),
                    AX.X, ALU.add)
                t10 = rp.tile([B, OL], F32, tag="t10")
                nc.scalar.activation(t10[:], sq0[:], ACTF.Sqrt, bias=eps_t[0:B])
                pl0 = rp.tile([B, OL], F32, tag="pl0")
                nc.vector.tensor_scalar_add(pl0[:], sq0[:], 1.0)
                dn0 = rp.tile([B, OL], F32, tag="dn0")
                nc.vector.tensor_tensor(dn0[:], pl0[:], t10[:], op=ALU.mult)
                rc0 = rp.tile([B, OL], F32, tag="rc0")
                nc.vector.reciprocal(rc0[:], dn0[:])
                fc0 = rp.tile([B, OL], F32, tag="fc0")
                nc.vector.tensor_tensor(fc0[:], sq0[:], rc0[:], op=ALU.mult)
                v0 = rp.tile([B, OL, V], F16, tag="v0")
                nc.vector.tensor_tensor(
                    v0[:], s0T[:].rearrange("p (o v) -> p o v", v=V),
                    fc0[:].unsqueeze(2).to_broadcast((B, OL, V)),
                    op=ALU.mult)
                nc.sync.dma_start(v0_dram.rearrange("b f -> b f"), v0[:].rearrange("b o v -> b (o v)"))

            bl_cur = None
            for it in range(2):
                # ---- vrep [128, OV]: replicate v rows across i_l ----
                vrep = rp.tile([128, OV], F16, tag="vrep", bufs=2)
                for b in range(BG):
                    dst = vrep[:].rearrange("(i l) f -> i l f", l=BG)[:, b, :]
                    if it == 0:
                        src = v0_dram[BG * g + b, :].partition_broadcast(IL)
                    else:
                        src = (v1_dram.rearrange("(b o) v -> b (o v)", o=OL)
                               [b, :].partition_broadcast(IL))
                    nc.sync.dma_start(dst, src)

                # ---- b-pass: bl += sum_v xh * vrep ----
                bl_new = blp.tile([128, NBLK, OL], F32, tag="bl")
                for (c0, c1) in tree_chunks:
                    nb = c1 - c0
                    tp = trp.tile([128, NTCH, OL, V], F16, tag="tp")
                    nc.vector.tensor_tensor(
                        tp[:, 0:nb],
                        xh[:, c0:c1].rearrange("p k (o v) -> p k o v", v=V),
                        vrep[:].rearrange("p (o v) -> p o v", v=V)
                        .unsqueeze(1).to_broadcast((128, nb, OL, V)),
                        op=ALU.mult)
                    tr1 = trp.tile([128, NTCH, OL, 8], F16, tag="tr1")
                    nc.vector.tensor_tensor(
                        tr1[:, 0:nb], tp[:, 0:nb, :, 0:8],
                        tp[:, 0:nb, :, 8:16], op=ALU.add)
                    tr2 = trp.tile([128, NTCH, OL, 4], F16, tag="tr2")
                    nc.vector.tensor_tensor(
                        tr2[:, 0:nb], tr1[:, 0:nb, :, 0:4],
                        tr1[:, 0:nb, :, 4:8], op=ALU.add)
                    tr3 = trp.tile([128, NTCH, OL, 2], F16, tag="tr3")
                    nc.vector.tensor_tensor(
                        tr3[:, 0:nb], tr2[:, 0:nb, :, 0:2],
                        tr2[:, 0:nb, :, 2:4], op=ALU.add)
                    if it == 0:
                        nc.vector.tensor_tensor(
                            bl_new[:, c0:c1], tr3[:, 0:nb, :, 0],
                            tr3[:, 0:nb, :, 1], op=ALU.add)
                    else:
                        tr4 = trp.tile([128, NTCH, OL], F32, tag="tr4")
                        nc.vector.tensor_tensor(
                            tr4[:, 0:nb], tr3[:, 0:nb, :, 0],
                            tr3[:, 0:nb, :, 1], op=ALU.add)
                        nc.vector.tensor_tensor(
                            bl_new[:, c0:c1], bl_cur[:, c0:c1],
                            tr4[:, 0:nb], op=ALU.add)
                bl_cur = bl_new

                # ---- softmax over o (cross-core denominator) ----
                el = rp.tile([128, NBLK, OL], F16, tag="el")
                nc.scalar.activation(
                    el[:].rearrange("p k o -> p (k o)"),
                    bl_cur[:].rearrange("p k o -> p (k o)"), ACTF.Exp)
                dl = rp.tile([128, NBLK], F32, tag="dl")
                nc.vector.tensor_reduce(dl[:], el[:], AX.X, ALU.add)
                dlb = rp.tile([128, NBLK], F32, tag="dlb")
                nc.vector.tensor_scalar_add(dlb[:], dl[:], dbias_t[:])
                nc.sync.dma_start(dl_dram, dlb[:])
                nc.gpsimd.collective_compute(
                    "AllReduce", ALU.add,
                    replica_groups=[list(range(NC))],
                    ins=[dl_dram.opt()], outs=[dg_dram.opt()])
                dg = rp.tile([128, NBLK], F32, tag="dg")
                nc.sync.dma_start(dg[:], dg_dram)
                drc = rp.tile([128, NBLK], F32, tag="drc")
                nc.vector.reciprocal(drc[:], dg[:])
                c16 = rp.tile([128, NBLK, OL], F16, tag="c16")
                nc.vector.tensor_tensor(
                    c16[:], el[:],
                    drc[:].unsqueeze(2).to_broadcast((128, NBLK, OL)),
                    op=ALU.mult)
                cm = rp.tile([128, NBLK, BG, OL], F16, tag="cm")
                nc.vector.tensor_tensor(
                    cm[:],
                    c16[:].unsqueeze(2).to_broadcast((128, NBLK, BG, OL)),
                    maskb_t[:].rearrange("p (b o) -> p b o", o=OL)
                    .unsqueeze(1).to_broadcast((128, NBLK, BG, OL)),
                    op=ALU.mult)

                # ---- s-pass: S[(b,o'),(o,v)] = sum_i cm^T xh ----
                pss = sps.tile([BG * OL, OV], F32, tag="spsum", bufs=2)
                for blk in range(NBLK):
                    nc.tensor.matmul(
                        pss[:], cm[:, blk].rearrange("p b o -> p (b o)"),
                        xh[:, blk],
                        start=(blk == 0), stop=(blk == NBLK - 1))

                # ---- diag extract + squash ----
                s_sb = rp.tile([BG * OL, OV], F32, tag="s_sb")
                nc.scalar.copy(s_sb[:], pss[:])
                sd = rp.tile([BG * OL, V], F32, tag="sd")
                for o in range(OL):
                    # strided-partition access: DMA only (engines need step-1)
                    nc.sync.dma_start(
                        sd[:].rearrange("(b o) v -> b o v", o=OL)[:, o, :],
                        s_sb[:].rearrange("(b o) (q v) -> b o q v", o=OL, v=V)
                        [:, o, o, :])
                p2s = rp.tile([BG * OL, V], F32, tag="p2s")
                nc.vector.tensor_tensor(p2s[:], sd[:], sd[:], op=ALU.mult)
                sqs = rp.tile([BG * OL, 1], F32, tag="sqs")
                nc.vector.tensor_reduce(
                    sqs[:], p2s[:].unsqueeze(1), AX.X, ALU.add)
                t1s = rp.tile([BG * OL, 1], F32, tag="t1s")
                nc.scalar.activation(t1s[:], sqs[:], ACTF.Sqrt, bias=eps_t[0:BG * OL])
                pls = rp.tile([BG * OL, 1], F32, tag="pls")
                nc.vector.tensor_scalar_add(pls[:], sqs[:], 1.0)
                dns = rp.tile([BG * OL, 1], F32, tag="dns")
                nc.vector.tensor_tensor(dns[:], pls[:], t1s[:], op=ALU.mult)
                rcs = rp.tile([BG * OL, 1], F32, tag="rcs")
                nc.vector.reciprocal(rcs[:], dns[:])
                fcs = rp.tile([BG * OL, 1], F32, tag="fcs")
                nc.vector.tensor_tensor(fcs[:], sqs[:], rcs[:], op=ALU.mult)
                if it == 0:
                    v1 = rp.tile([BG * OL, V], F16, tag="v1")
                    nc.vector.tensor_scalar_mul(v1[:], sd[:], fcs[:])
                    nc.sync.dma_start(v1_dram, v1[:])
                else:
                    vf = rp.tile([BG * OL, V], F32, tag="vf")
                    nc.vector.tensor_scalar_mul(vf[:], sd[:], fcs[:])
                    nc.sync.dma_start(
                        v_out[BG * g:BG * (g + 1)]
                        .rearrange("b o v -> (b o) v"), vf[:])

    nc.compile()
    return nc


_NC_CACHE = {}


def _prep_inputs(x, conv1_w, conv1_b, pconv_w, pconv_b, dense_w):
    x = np.asarray(x, np.float32)
    conv1_w = np.asarray(conv1_w, np.float32)
    conv1_b = np.asarray(conv1_b, np.float32)
    pconv_w = np.asarray(pconv_w, np.float32)
    pconv_b = np.asarray(pconv_b, np.float32)
    dense_w = np.asarray(dense_w, np.float32)

    xp = np.zeros((B, 3, IMGP), np.float16)
    xp[:, :, :IMG] = x.reshape(B, 3, IMG).astype(np.float16)
    w1 = (conv1_w.transpose(3, 1, 2, 0).reshape(11, 33, 256)
          .astype(np.float16))
    w2 = (pconv_w.transpose(2, 3, 1, 0).reshape(81, 256, 256)
          .astype(np.float16))
    dwp = np.zeros((NC * OL, NI, V, D), np.float32)
    dwp[:101] = dense_w

    il_idx = np.arange(128) // D            # i_l of partition row
    maskd = (il_idx[:, None] == (np.arange(128) // BG)[None, :]).astype(np.float16)
    b_idx = np.arange(128) % BG             # b of partition row
    maskb = (b_idx[:, None] == (np.arange(8 * OL) // OL)[None, :]).astype(np.float16)
    ident = np.eye(128, dtype=np.float16)

    in_maps = []
    for k in range(NC):
        sl = dwp[OL * k: OL * (k + 1)]                      # [13,2592,16,8]
        wd = (sl.transpose(1, 3, 0, 2)                      # [2592,8,13,16]
              .reshape(NBLK, IL, D, OL * V)
              .reshape(NBLK, 128, OL * V).astype(np.float16))
        dbias = np.full((128, 1), -3.0 if k == NC - 1 else 0.0, np.float32)
        in_maps.append(dict(
            x=xp[BL * k: BL * (k + 1)],
            w1=w1, b1=conv1_b, w2=w2, b2=pconv_b,
            wd=np.ascontiguousarray(wd),
            maskd=maskd, maskb=maskb, ident=ident, dbias=dbias,
        ))
    return in_maps


def _run(inputs, trace=False, trace_kwargs=None):
    in_maps = _prep_inputs(**inputs)
    if "nc" not in _NC_CACHE:
        _NC_CACHE["nc"] = _build()
    nc = _NC_CACHE["nc"]
    res = run_bass_kernel_spmd(nc, in_maps, list(range(NC)), trace=trace,
                               **(trace_kwargs or {}))
    v = np.concatenate([res.results[k]["v_out"] for k in range(NC)], axis=1)
    return v[:, :101, :].astype(np.float32), res


def kernel(**inputs):
    out, _ = _run(inputs, trace=False)
    return out
